# revision 9
# baseline (speedup 1.0000x reference)
"""Trainium2 Bass kernel for MultiHeadedAttention (gnn_message_passing variant).

Computes, for full inputs (N=4096, d_model=512, H=4, d_k=64):
    q  = query @ Wq.T + bq   (per head, scaled by 1/sqrt(d_k) -- folded into Wq)
    k  = key   @ Wk.T + bk
    qv = query @ Wqv.T + bqv
    kv = key   @ Wkv.T + bkv
    s  = q @ k.T  (+ additive mask: -57344 where mask==0)
    p  = softmax(s)          [h, N, N]  (exp without row-max: |s| <= ~10)
    x1 = ELU(p @ kv)         [N, 256]
    x2 = ELU(p.T @ qv)       [N, 256]  (all-reduce over query slabs)

Sharding: query rows split into 8 slabs of 512, one per NeuronCore. Each core
computes p/x1 for its slab and an x2 partial; x2 partials are AllReduced on
device, each core then emits its 512-row slab of x2.

Per core the score matrix is computed in both orientations:
  - [q-part, k-free]: softmax rows on the free axis -> p output + x2 matmuls
  - [k-part, q-free]: exp'd scores (eT) feed the x1 matmul (contraction over k
    must sit on the partition axis)
The additive mask enters the PSUM accumulation as an identity-stationary
matmul with an fp8(e5m2) mask operand (0 / -57344), in both orientations.
"""

import math
from contextlib import ExitStack

import numpy as np
import ml_dtypes

import concourse.bass as bass
import concourse.tile as tile
from concourse import bacc, mybir
from concourse.bass_utils import run_bass_kernel_spmd

F32 = mybir.dt.float32
F32R = mybir.dt.float32r
BF16 = mybir.dt.bfloat16
F8 = mybir.dt.float8e5

NCORES = 8
N = 4096          # sequence / node count
D = 512           # d_model
O = 256           # out_dim (H * DK)
H = 4
DK = 64
S = N // NCORES   # query rows per core (512)
QC = S // 128     # q chunks per core (4)
KB = N // 128     # k blocks (32)
NEG = -57344.0    # additive mask value (max-magnitude fp8e5m2)


def r32(ap):
    return ap.bitcast(F32R)


def _build():
    nc = bacc.Bacc("TRN2", target_bir_lowering=False, debug=False,
                   num_devices=NCORES)
    dt_in = {}

    def din(name, shape, dt=F32):
        dt_in[name] = nc.dram_tensor(name, shape, dt, kind="ExternalInput").ap()
        return dt_in[name]

    qT_in = din("qT_in", [D, S], F32R)            # query.T slab
    kT_in = din("kT_in", [D, N], F32R)            # key.T full
    wq_in = din("wq_in", [D, O], F32R)            # (Wq/8).T
    wk_in = din("wk_in", [D, O], F32R)
    wqv_in = din("wqv_in", [D, O], F32R)
    wkv_in = din("wkv_in", [D, O], F32R)
    bq_in = din("bq_in", [128, 2])          # per-partition bias, col=o-half
    bk_in = din("bk_in", [128, 2])
    bqv_in = din("bqv_in", [1, O], F32R)          # bias rows for K=1 matmul
    bkv_in = din("bkv_in", [1, O], F32R)
    ones_in = din("ones_in", [1, 128], F32R)
    id8_in = din("id8_in", [128, 128], F8)
    idf_in = din("idf_in", [128, 128])
    lmq_in = din("lmq_in", [S, N], F8)      # additive logmask rows slab
    lmt_in = din("lmt_in", [N, S], F8)      # transposed logmask, cols slab

    p_out = nc.dram_tensor("p_out", [H, S, N], F32R, kind="ExternalOutput").ap()
    x1_out = nc.dram_tensor("x1_out", [S, O], F32, kind="ExternalOutput").ap()
    x2_out = nc.dram_tensor("x2_out", [S, O], F32, kind="ExternalOutput").ap()

    with tile.TileContext(nc) as tc, ExitStack() as ctx:
        Exp = mybir.ActivationFunctionType.Exp
        Ident = mybir.ActivationFunctionType.Identity
        Copy = mybir.ActivationFunctionType.Copy

        const = ctx.enter_context(tc.tile_pool(name="const", bufs=1))
        core = ctx.enter_context(tc.tile_pool(name="core", bufs=1))
        psc = ctx.enter_context(tc.tile_pool(name="psc", bufs=2, space="PSUM"))
        pacc = ctx.enter_context(tc.tile_pool(name="pacc", bufs=4, space="PSUM"))
        dram = ctx.enter_context(tc.tile_pool(name="dram", bufs=1, space="DRAM"))

        # ---- constants ----
        id8 = const.tile([128, 128], F8)
        nc.sync.dma_start(id8[:], id8_in[:])
        idf = const.tile([128, 128], F32)
        nc.sync.dma_start(idf[:], idf_in[:])
        ones1 = const.tile([1, 128], F32R)
        nc.sync.dma_start(ones1[:], ones_in[:])
        bq2 = const.tile([128, 2], F32)
        nc.sync.dma_start(bq2[:], bq_in[:])
        bk2 = const.tile([128, 2], F32)
        nc.sync.dma_start(bk2[:], bk_in[:])
        bqv1 = const.tile([1, O], F32R)
        nc.sync.dma_start(bqv1[:], bqv_in[:])
        bkv1 = const.tile([1, O], F32R)
        nc.sync.dma_start(bkv1[:], bkv_in[:])

        # ---- persistent operand tiles (alive through the whole kernel) ----
        kT = [core.tile([128, N], F32R, name=f"kT{j}") for j in range(2)]
        qT = [core.tile([128, S], F32R, name=f"qTt{j}") for j in range(2)]
        kv = [core.tile([128, O], BF16, name=f"kv{b}") for b in range(KB)]
        qv = [core.tile([128, O], F32R, name=f"qv{c}") for c in range(QC)]

        # ---- projections (scoped input pool) ----
        with tc.tile_pool(name="proj", bufs=1) as proj:
            keyT = [proj.tile([128, N], F32R, name=f"keyT{d}") for d in range(4)]
            quT = [proj.tile([128, S], F32R, name=f"quT{d}") for d in range(4)]
            wq = [proj.tile([128, O], F32R, name=f"wq{d}") for d in range(4)]
            wk = [proj.tile([128, O], F32R, name=f"wk{d}") for d in range(4)]
            wqv = [proj.tile([128, O], F32R, name=f"wqv{d}") for d in range(4)]
            wkv = [proj.tile([128, O], F32R, name=f"wkv{d}") for d in range(4)]
            for d in range(4):
                sl = slice(d * 128, (d + 1) * 128)
                nc.sync.dma_start(keyT[d][:], kT_in[sl, :])
                nc.sync.dma_start(quT[d][:], qT_in[sl, :])
                nc.sync.dma_start(wq[d][:], wq_in[sl, :])
                nc.sync.dma_start(wk[d][:], wk_in[sl, :])
                nc.sync.dma_start(wqv[d][:], wqv_in[sl, :])
                nc.sync.dma_start(wkv[d][:], wkv_in[sl, :])

            # kT[o, n] (2 o-halves x 8 n-chunks) and qT[o, s]
            for j in range(2):
                osl = slice(j * 128, (j + 1) * 128)
                for nb in range(8):
                    nsl = slice(nb * 512, (nb + 1) * 512)
                    ps = psc.tile([128, 1024], F32, tag="sc")
                    for d in range(4):
                        nc.tensor.matmul(ps[:, 0:512], wk[d][:, osl],
                                         keyT[d][:, nsl],
                                         start=(d == 0), stop=(d == 3))
                    nc.scalar.activation(kT[j][:, nsl], ps[:, 0:512], Ident,
                                         bias=bk2[:, j:j + 1], scale=1.0)
                ps = psc.tile([128, 1024], F32, tag="sc")
                for d in range(4):
                    nc.tensor.matmul(ps[:, 0:512], wq[d][:, osl],
                                     quT[d][:, :],
                                     start=(d == 0), stop=(d == 3))
                nc.scalar.activation(qT[j][:, :], ps[:, 0:512], Ident,
                                     bias=bq2[:, j:j + 1], scale=1.0)

            # qv[s, o] natural (4 q-chunks)
            for c in range(QC):
                csl = slice(c * 128, (c + 1) * 128)
                ps = psc.tile([128, 1024], F32, tag="sc")
                for d in range(4):
                    nc.tensor.matmul(ps[:, 0:O], quT[d][:, csl],
                                     wqv[d][:, :],
                                     start=(d == 0), stop=False)
                nc.tensor.matmul(ps[:, 0:O], ones1[:, 0:128],
                                 bqv1[:, :], start=False, stop=True)
                nc.vector.tensor_copy(qv[c][:], ps[:, 0:O])

            # kv[n, o] natural (32 k-blocks), bf16
            for b in range(KB):
                bsl = slice(b * 128, (b + 1) * 128)
                ps = psc.tile([128, 1024], F32, tag="sc")
                for d in range(4):
                    nc.tensor.matmul(ps[:, 0:O], keyT[d][:, bsl],
                                     wkv[d][:, :],
                                     start=(d == 0), stop=False)
                nc.tensor.matmul(ps[:, 0:O], ones1[:, 0:128],
                                 bkv1[:, :], start=False, stop=True)
                nc.vector.tensor_copy(kv[b][:], ps[:, 0:O])

        # ---- pools for the main pipeline (reuse space freed by proj) ----
        persist = ctx.enter_context(tc.tile_pool(name="persist", bufs=1))
        stage = ctx.enter_context(tc.tile_pool(name="stage", bufs=1))
        lmq = [persist.tile([128, N], F8, name=f"lmq{c}") for c in range(QC)]
        lmt = [persist.tile([128, S], F8, name=f"lmt{b}") for b in range(KB)]
        for c in range(QC):
            nc.sync.dma_start(lmq[c][:], lmq_in[c * 128:(c + 1) * 128, :])
        for b in range(KB):
            nc.sync.dma_start(lmt[b][:], lmt_in[b * 128:(b + 1) * 128, :])
        # eT: exp'd transposed scores for one head, block b at cols [b*S, b*S+S)
        eT = persist.tile([128, KB * S], BF16)
        # x1T stash: head h at cols [h*S, h*S+S), partitions 0..63
        x1s = persist.tile([128, H * S], F32)
        # per (head, chunk) softmax sums and reciprocals
        rs = [persist.tile([128, 4], F32, name=f"rs{i}") for i in range(H * QC)]
        rcp = [persist.tile([128, 1], F32, name=f"rcp{i}") for i in range(H * QC)]

        # ---- x2 bounce buffers for the all-reduce ----
        x2b_in = dram.tile([O, N], F32)
        x2b_out = dram.tile([O, N], F32, addr_space="Shared")

        # ---- main per-head pipeline ----
        for h in range(H):
            j, po = h // 2, 64 * (h % 2)
            hsl = slice(po, po + 64)

            # T phase: eT[k, q] = exp(sT + maskT), then x1T accumulation
            for g in range(KB // 2):
                ps = psc.tile([128, 1024], F32, tag="sc", name=f"tps{h}_{g}")
                for u in range(2):
                    b = 2 * g + u
                    usl = slice(u * 512, u * 512 + 512)
                    nc.tensor.matmul(ps[:, usl],
                                     kT[j][hsl, b * 128:(b + 1) * 128],
                                     qT[j][hsl, :], start=True, stop=False)
                    nc.tensor.matmul(ps[:, usl], id8[:], lmt[b][:],
                                     start=False, stop=True)
                nc.scalar.activation(eT[:, g * 1024:(g + 1) * 1024], ps[:], Exp)
            x1p = pacc.tile([128, 512], F32, tag="acc", name=f"x1p{h}")
            for b in range(KB):
                nc.tensor.matmul(x1p[0:64, :], kv[b][:, h * 64:h * 64 + 64],
                                 eT[:, b * S:(b + 1) * S],
                                 start=(b == 0), stop=(b == KB - 1))
            nc.vector.tensor_copy(x1s[0:64, h * S:(h + 1) * S], x1p[0:64, :])

            # q phase: p rows + rowsums, then x2T accumulation in two waves
            # (wave A: k-blocks 0..3 from the first p-half of each chunk,
            #  wave B: k-blocks 4..7 from the retained second halves)
            x2A = [pacc.tile([128, 512], F32, tag="acc", name=f"x2A{h}_{g}")
                   for g in range(4)]
            ph1 = []  # retained second halves, one per chunk
            for c in range(QC):
                i = h * QC + c
                csl = slice(c * 128, (c + 1) * 128)
                phalf = []
                for half in range(2):
                    ph = stage.tile([128, 2048], F32R, tag="pstage", bufs=6,
                                    name=f"ph{h}_{c}_{half}")
                    phalf.append(ph)
                    for q4 in range(2):
                        ps = psc.tile([128, 1024], F32, tag="sc",
                                      name=f"qps{h}_{c}_{half}_{q4}")
                        for u in range(2):
                            nb = half * 4 + q4 * 2 + u
                            nsl = slice(nb * 512, (nb + 1) * 512)
                            nc.tensor.matmul(ps[:, u * 512:u * 512 + 512],
                                             qT[j][hsl, csl],
                                             kT[j][hsl, nsl],
                                             start=True, stop=False)
                            nc.tensor.matmul(ps[:, u * 512:u * 512 + 512],
                                             id8[:], lmq[c][:, nsl],
                                             start=False, stop=True)
                        nc.scalar.activation(
                            ph[:, q4 * 1024:(q4 + 1) * 1024], ps[:], Exp,
                            accum_out=rs[i][:, half * 2 + q4:half * 2 + q4 + 1])
                nc.vector.tensor_reduce(
                    rcp[i][:], rs[i][:], mybir.AxisListType.X,
                    mybir.AluOpType.add)
                nc.vector.reciprocal(rcp[i][:], rcp[i][:])
                for half in range(2):
                    ph = phalf[half]
                    nc.vector.tensor_scalar_mul(ph[:], ph[:], rcp[i][:])
                    nc.sync.dma_start(
                        p_out[h, csl, half * 2048:(half + 1) * 2048], ph[:])
                ph1.append(phalf[1])
                for g in range(4):
                    nc.tensor.matmul(
                        x2A[g][0:64, :], qv[c][:, h * 64:h * 64 + 64],
                        phalf[0][:, g * 512:(g + 1) * 512],
                        start=(c == 0), stop=(c == QC - 1))

            def x2_evict(tiles, koff):
                for g in range(4):
                    x2s = stage.tile([128, 512], F32, tag="x2s", bufs=2,
                                     name=f"x2s{h}_{koff}_{g}")
                    nc.vector.tensor_copy(x2s[0:64, :], tiles[g][0:64, :])
                    nc.sync.dma_start(
                        x2b_in[h * 64:h * 64 + 64,
                               (koff + g) * 512:(koff + g + 1) * 512],
                        x2s[0:64, :])

            x2_evict(x2A, 0)
            x2B = [pacc.tile([128, 512], F32, tag="acc", name=f"x2B{h}_{g}")
                   for g in range(4)]
            for c in range(QC):
                for g in range(4):
                    nc.tensor.matmul(
                        x2B[g][0:64, :], qv[c][:, h * 64:h * 64 + 64],
                        ph1[c][:, g * 512:(g + 1) * 512],
                        start=(c == 0), stop=(c == QC - 1))
            x2_evict(x2B, 4)

        # ---- x2 all-reduce over query slabs ----
        nc.gpsimd.collective_compute(
            "AllReduce", mybir.AluOpType.add,
            replica_groups=[list(range(NCORES))],
            ins=[x2b_in.opt()], outs=[x2b_out.opt()])

        # ---- finalize x1: transpose + scale(1/Z) + ELU ----
        def elu_store(dst, src_tile):
            # src_tile: [128, O] f32 pre-activation; writes ELU(src) to dst
            tmin = stage.tile([128, O], F32, bufs=2, name="tmin")
            nc.vector.tensor_scalar_min(tmin[:], src_tile[:], 0.0)
            texp = stage.tile([128, O], F32, bufs=2, name="texp")
            nc.scalar.activation(texp[:], tmin[:], Exp)
            trel = stage.tile([128, O], F32, bufs=2, name="trel")
            nc.vector.tensor_scalar(trel[:], src_tile[:], 0.0, -1.0,
                                    op0=mybir.AluOpType.max,
                                    op1=mybir.AluOpType.add)
            tout = stage.tile([128, O], F32, bufs=2, name="tout")
            nc.vector.tensor_add(tout[:], trel[:], texp[:])
            nc.sync.dma_start(dst, tout[:])

        for c in range(QC):
            x1n = stage.tile([128, O], F32, tag="x1n", bufs=2, name=f"x1n{c}")
            for h in range(H):
                tp = pacc.tile([128, 512], F32, tag="acc", name=f"tpx1_{c}_{h}")
                nc.tensor.transpose(tp[:, 0:64],
                                    x1s[0:64, h * S + c * 128:h * S + (c + 1) * 128],
                                    idf[0:64, 0:64])
                nc.scalar.activation(x1n[:, h * 64:h * 64 + 64], tp[:, 0:64],
                                     Copy, scale=rcp[h * QC + c][:])
            elu_store(x1_out[c * 128:(c + 1) * 128, :], x1n)

        # ---- finalize x2: extract this core's k-slab, transpose, ELU ----
        core = nc.gpsimd.partition_id()
        x2f = [stage.tile([128, 512], F32, tag="x2f", bufs=2, name=f"x2f{t}")
               for t in range(2)]
        for t in range(2):
            nc.gpsimd.dma_start(x2f[t][:],
                                x2b_out[t * 128:(t + 1) * 128, bass.ts(core, S)])
        for c in range(QC):
            x2n = stage.tile([128, O], F32, tag="x1n", bufs=2, name=f"x2n{c}")
            for t in range(2):
                tp = pacc.tile([128, 512], F32, tag="acc", name=f"tpx2_{c}_{t}")
                nc.tensor.transpose(tp[:, 0:128],
                                    x2f[t][:, c * 128:(c + 1) * 128], idf[:])
                nc.vector.tensor_copy(x2n[:, t * 128:(t + 1) * 128],
                                      tp[:, 0:128])
            elu_store(x2_out[c * 128:(c + 1) * 128, :], x2n)

    nc.compile()
    return nc


_NC_CACHE = {}


def get_nc():
    if "nc" not in _NC_CACHE:
        _NC_CACHE["nc"] = _build()
    return _NC_CACHE["nc"]


def make_in_maps(query, key, mask, Wq, bq, Wk, bk, Wqv, bqv, Wkv, bkv):
    f8 = ml_dtypes.float8_e5m2
    s = 1.0 / math.sqrt(DK)
    qT = np.ascontiguousarray(query.T.astype(np.float32))
    kT = np.ascontiguousarray(key.T.astype(np.float32))
    wq = np.ascontiguousarray((Wq * s).T.astype(np.float32))
    wk = np.ascontiguousarray(Wk.T.astype(np.float32))
    wqv = np.ascontiguousarray(Wqv.T.astype(np.float32))
    wkv = np.ascontiguousarray(Wkv.T.astype(np.float32))
    bq2 = np.ascontiguousarray((bq * s).reshape(2, 128).T.astype(np.float32))
    bk2 = np.ascontiguousarray(bk.reshape(2, 128).T.astype(np.float32))
    bqv1 = bqv.reshape(1, O).astype(np.float32)
    bkv1 = bkv.reshape(1, O).astype(np.float32)
    ones1 = np.ones((1, 128), np.float32)
    id8 = np.eye(128, dtype=np.float32).astype(f8)
    idf = np.eye(128, dtype=np.float32)
    logm = np.where(mask != 0, 0.0, NEG).astype(f8)          # [N, N]
    logmT = np.ascontiguousarray(logm.T)

    in_maps = []
    for i in range(NCORES):
        sl = slice(i * S, (i + 1) * S)
        in_maps.append({
            "qT_in": np.ascontiguousarray(qT[:, sl]),
            "kT_in": kT,
            "wq_in": wq, "wk_in": wk, "wqv_in": wqv, "wkv_in": wkv,
            "bq_in": bq2, "bk_in": bk2, "bqv_in": bqv1, "bkv_in": bkv1,
            "ones_in": ones1, "id8_in": id8, "idf_in": idf,
            "lmq_in": np.ascontiguousarray(logm[sl, :]),
            "lmt_in": np.ascontiguousarray(logmT[:, sl]),
        })
    return in_maps


def kernel(query, key, mask, Wq, bq, Wk, bk, Wqv, bqv, Wkv, bkv):
    nc = get_nc()
    in_maps = make_in_maps(query, key, mask, Wq, bq, Wk, bk,
                           Wqv, bqv, Wkv, bkv)
    res = run_bass_kernel_spmd(nc, in_maps, core_ids=list(range(NCORES)))
    p = np.concatenate([r["p_out"] for r in res.results], axis=1)
    x1 = np.concatenate([r["x1_out"] for r in res.results], axis=0)
    x2 = np.concatenate([r["x2_out"] for r in res.results], axis=0)
    return x1, x2, p
